# revision 44
# baseline (speedup 1.0000x reference)
"""Trainium2 Bass kernel: quantized BasicBlock (quant-conv3x3 -> bn -> relu ->
quant-conv3x3 -> bn -> +residual -> relu).

Sharding: data-parallel over the batch dim of x across 8 NeuronCores (8 images
per core).  Weight quantization (centroid/deviation pipeline) is replicated on
every core, computed on-device.

Algorithm: 1D Winograd F(2,3) along the H axis.  Each 3x3 conv becomes, per
output-row-pair r', 4 "spectral" components m_u = sum_{ci,dx} wU_u[ci,co,dx] *
v_u[ci, r', x+dx] where
    v0 = d0 - d2,  v1 = d1 + d2,  v2 = d2 - d1,  v3 = d1 - d3   (d_a = row 2r'+a)
    u0 = g0, u1 = (g0+g1+g2)/2, u2 = (g0-g1+g2)/2, u3 = g2      (g_dy = w taps)
    out(2r')   = m0 + m1 + m2
    out(2r'+1) = m1 - m2 - m3
This does 24 matmuls per (image, co-chunk) site instead of 36 (1.5x less PE
work); the cheap row combines run on DVE in fp16, the PSUM->SBUF copies (with
the BN scale folded) on ScalarE.

Math notes:
  - jnp.round (round-half-even) is implemented with the fp32 magic-number
    trick: rne(v) = (v + 1.5*2^23) - 1.5*2^23 for |v| < 2^22.
  - Quantized weights are integer "levels" dev+cent = k/8 with |k| < 2048,
    exactly representable in fp16.  The Winograd u1/u2 combos are k/16 halves,
    exact in fp16 up to |u| < 128 (overflow beyond is ~5-sigma rare and only
    loses the 1/16 fraction).  The global scale `step` is folded into the BN
    scale vector, so matmuls run in fp16 with fp32 PSUM accumulation.
"""

import sys

for _p in ("/opt/trn_rl_repo",):
    if _p not in sys.path:
        sys.path.insert(0, _p)

from contextlib import ExitStack

import numpy as np

import concourse.tile as tile
from concourse import bacc, mybir
from concourse.bass_utils import run_bass_kernel_spmd
from concourse.masks import make_identity

P = 128
B, C, H, W = 64, 256, 28, 28
NCORES = 8
BPC = B // NCORES          # images per core
CK = C // P                # channel chunks (2)
TAPS = 9
HP, WP = H + 2, W + 4      # padded spatial 30x32 (x padded to 32 for alignment)
XO = 2                     # x data starts at col XO; zero-pad cols XO-1, XO+W
R2 = H // 2                # winograd row-pair groups (14)
NN = R2 * W                # matmul free dim (392)
F32 = mybir.dt.float32
F16 = mybir.dt.float16

MAGIC = 12582912.0         # 1.5 * 2**23  (fp32 RNE round-to-int trick)
HALF_LVLS = 127.0
LV = 8.0                   # 2**(NUM_BITS-1)
CSTEP = HALF_LVLS / LV     # 15.875
DEVW = 0.5 * HALF_LVLS     # 63.5
BN_EPS = 1e-5

AF = mybir.ActivationFunctionType
OP = mybir.AluOpType
AX = mybir.AxisListType


def _emit(nc, tc, ctx, td):
    """Emit the whole per-core program.  td: dict of DRAM tensor handles."""
    const = ctx.enter_context(tc.tile_pool(name="const", bufs=1))
    bnp = ctx.enter_context(tc.tile_pool(name="bnp", bufs=2))
    wbig = ctx.enter_context(tc.tile_pool(name="wbig", bufs=1))
    whalf = ctx.enter_context(tc.tile_pool(name="whalf", bufs=2))
    wqp = ctx.enter_context(tc.tile_pool(name="wqp", bufs=2))
    wtp = ctx.enter_context(tc.tile_pool(name="wtp", bufs=1))
    wup = ctx.enter_context(tc.tile_pool(name="wup", bufs=1))
    wsp = ctx.enter_context(tc.tile_pool(name="wsp", bufs=3))
    tpp = ctx.enter_context(tc.tile_pool(name="tpp", bufs=1, space="PSUM"))
    psp = ctx.enter_context(tc.tile_pool(name="psp", bufs=7, space="PSUM"))
    pxf = ctx.enter_context(tc.tile_pool(name="pxf", bufs=2))
    pxp = ctx.enter_context(tc.tile_pool(name="pxp", bufs=2))
    pxb = ctx.enter_context(tc.tile_pool(name="pxb", bufs=7))
    phh = ctx.enter_context(tc.tile_pool(name="phh", bufs=6))
    pvv = ctx.enter_context(tc.tile_pool(name="pvv", bufs=4))
    pmu = ctx.enter_context(tc.tile_pool(name="pmu", bufs=6))
    ptt = ctx.enter_context(tc.tile_pool(name="ptt", bufs=8))
    pyy = ctx.enter_context(tc.tile_pool(name="pyy", bufs=2))

    ident16 = const.tile([P, P], F16, name="ident16", tag="ident16")
    make_identity(nc, ident16)
    ident32 = const.tile([P, P], F32, name="ident32", tag="ident32")
    make_identity(nc, ident32)
    ones32 = const.tile([1, P], F32, name="ones32", tag="ones32")
    nc.gpsimd.memset(ones32[:], 1.0)
    magicv = const.tile([P, 1], F32, name="magicv", tag="magicv")
    nc.gpsimd.memset(magicv[:], MAGIC)
    # warm the ScalarE activation tables during the initial DMA wait so the
    # one-time ACT_TABLE_LOADs don't block the quant chain later
    scr = const.tile([P, 1], F32, name="scr", tag="scr")
    nc.scalar.activation(scr[:], magicv[:], AF.Sqrt)
    nc.scalar.activation(scr[:], magicv[:], AF.Relu)

    wT = {}      # wT[j][k] : [P(ci), CK(m), TAPS, P(co)] fp16 (dy,dx taps)
    wU = {}      # wU[j][k] : [P(ci), CK(m), 2(u1,u2), 3(dx), P(co)] fp16
    inv_s = {}   # BN scale with quant step folded in: [P, CK]
    ninv_s = {}  # -inv_s
    bvec = {}    # BN bias: [P, CK]
    _w32 = {}
    _wq = {}
    _istep = {}
    _inv = {}
    _step = {}

    # ---------------- image loads ------------------------------------------
    x_view = td["x"].ap().rearrange("b (c p) h w -> b p c h w", p=P)
    y_view = td["y"].ap().rearrange("b (c p) h w -> b p c h w", p=P)
    xp_t = [None] * BPC
    xb_t = [None] * BPC
    h_t = [None] * BPC
    v1_t = [None] * BPC
    v2_t = [None] * BPC

    xf_t = [None] * BPC

    def load_dma(i):
        xf = pxf.tile([P, CK, H, W], F32, name=f"xf{i}", tag="xf")
        nc.sync.dma_start(xf[:], x_view[i])
        xf_t[i] = xf

    def prep_x(i):
        """Pad-copy + residual prep for image i (ScalarE + gpsimd memsets)."""
        xf = xf_t[i]
        xp = pxp.tile([P, CK, HP, WP], F16, name=f"xp{i}", tag="xp")
        # zero only the borders; the interior is fully overwritten
        nc.gpsimd.memset(xp[:, :, 0:1, :], 0.0)
        nc.gpsimd.memset(xp[:, :, HP - 1 :, :], 0.0)
        nc.gpsimd.memset(xp[:, :, 1 : HP - 1, 0:XO], 0.0)
        nc.gpsimd.memset(xp[:, :, 1 : HP - 1, XO + W :], 0.0)
        nc.scalar.copy(xp[:, :, 1 : 1 + H, XO : XO + W], xf[:])
        # xb = x + bn2 bias (residual with the conv2 bias pre-added), fp16
        xb = pxb.tile([P, CK, H, W], F16, name=f"xb{i}", tag="xb")
        for c in range(CK):
            nc.scalar.activation(
                xb[:, c], xf[:, c], AF.Identity, bias=bvec[2][:, c : c + 1]
            )
        xp_t[i], xb_t[i] = xp, xb

    # ---------------- per-weight quantization ------------------------------
    def quant_dma(j):
        """Issue weight DMAs (sync engine only — no compute-engine stalls)."""
        w32 = wbig.tile([P, CK, C, TAPS], F32, name=f"w32_{j}", tag="wbig")
        wsrc = td[f"w{j}"].ap().rearrange("(c p) ci kh kw -> p c ci (kh kw)", p=P)
        for c in range(CK):
            for k in range(CK):
                ks = slice(k * P, (k + 1) * P)
                nc.sync.dma_start(w32[:, c, ks, :], wsrc[:, c, ks, :])
        _w32[j] = w32

    def quant_absmax(j):
        """Global absmax -> step/istep (reduces on DVE, combine on PE)."""
        w32 = _w32[j]
        pmq = []
        for c in range(CK):
            for k in range(CK):
                ks = slice(k * P, (k + 1) * P)
                ph = bnp.tile([P, 1], F32, name=f"pmq{j}_{c}_{k}", tag="pmq")
                nc.vector.tensor_reduce(
                    ph[:], w32[:, c, ks, :], axis=AX.XY, op=OP.max,
                    apply_absolute_value=True,
                )
                pmq.append(ph)
        pa = bnp.tile([P, 1], F32, name=f"pa{j}", tag="pa")
        nc.vector.tensor_max(pa[:], pmq[0][:], pmq[1][:])
        pb = bnp.tile([P, 1], F32, name=f"pb{j}", tag="pb")
        nc.vector.tensor_max(pb[:], pmq[2][:], pmq[3][:])
        pm = bnp.tile([P, 1], F32, name=f"pm{j}", tag="pm")
        nc.vector.tensor_max(pm[:], pa[:], pb[:])
        # cross-partition max via PE: transpose [128,1]->[1,128], reduce,
        # then broadcast back with a K=1 ones matmul (gpsimd ucode is ~10us)
        pmt = tpp.tile([1, P], F32, name=f"pmt{j}", tag="tp")
        nc.tensor.transpose(pmt[:], pm[:], ident32[:])
        sm = bnp.tile([1, 1], F32, name=f"sm{j}", tag="sm")
        nc.vector.tensor_reduce(sm[:], pmt[:], axis=AX.X, op=OP.max)
        pmb = tpp.tile([P, 1], F32, name=f"pmb{j}", tag="tp")
        nc.tensor.matmul(pmb[:], ones32[:], sm[:])
        pmax = bnp.tile([P, 1], F32, name=f"pmax{j}", tag="pmax")
        nc.vector.tensor_copy(pmax[:], pmb[:])
        step = const.tile([P, 1], F32, name=f"step{j}", tag=f"step{j}")
        nc.vector.tensor_scalar_mul(step[:], pmax[:], 1.0 / HALF_LVLS)
        _step[j] = step
        rmax = bnp.tile([P, 1], F32, name=f"rmax{j}", tag="rmax")
        nc.vector.reciprocal(rmax[:], pmax[:])
        istep = const.tile([P, 1], F32, name=f"istep{j}", tag=f"istep{j}")
        nc.vector.tensor_scalar_mul(istep[:], rmax[:], HALF_LVLS)
        _istep[j] = istep
        if j == 1:
            i16 = const.tile([P, 1], F16, name="istep16", tag="istep16")
            nc.vector.tensor_copy(i16[:], istep[:])
            _istep16[0] = i16
        # fold step into BN scale: inv_s = inv * step (and its negation, used
        # to fold the m3 subtraction into a scalar_tensor_tensor)
        ivs = const.tile([P, CK], F32, name=f"ivs{j}", tag=f"ivs{j}")
        nc.vector.tensor_scalar_mul(ivs[:], _inv[j][:], _step[j][:, 0:1])
        inv_s[j] = ivs
        nvs = const.tile([P, CK], F32, name=f"nvs{j}", tag=f"nvs{j}")
        nc.vector.tensor_scalar_mul(nvs[:], ivs[:], -1.0)
        ninv_s[j] = nvs

        wT[j] = []
        wU[j] = []
        for k in range(CK):
            wt = wtp.tile([P, CK, TAPS, P], F16, name=f"wT{j}_{k}", tag=f"wT{j}_{k}")
            wT[j].append(wt)
            wu = wup.tile([P, CK, 2, 3, P], F16, name=f"wU{j}_{k}", tag=f"wU{j}_{k}")
            wU[j].append(wu)

    def bn_prep(j):
        """BN vector prep.  Contiguous [1,256] row loads (one descriptor each
        — the [128,2] gather form is 256 tiny descriptors, ~10us), math on one
        partition, then redistribute to [128,2] via K=1 PE matmuls."""
        gv = bnp.tile([1, C], F32, name=f"gv{j}", tag=f"gv{j}")
        bev = bnp.tile([1, C], F32, name=f"bev{j}", tag=f"bev{j}")
        muv = bnp.tile([1, C], F32, name=f"muv{j}", tag=f"muv{j}")
        vav = bnp.tile([1, C], F32, name=f"vav{j}", tag=f"vav{j}")
        nc.sync.dma_start(gv[:], td[f"gamma{j}"].ap().unsqueeze(0))
        nc.sync.dma_start(bev[:], td[f"beta{j}"].ap().unsqueeze(0))
        nc.sync.dma_start(muv[:], td[f"mean{j}"].ap().unsqueeze(0))
        nc.sync.dma_start(vav[:], td[f"var{j}"].ap().unsqueeze(0))

        # redistribute the raw rows -> [P, 4, CK] via K=1 PE matmuls FIRST;
        # single-partition DVE ops are ~20x slower than full-width ones, so
        # all the math happens after the spread.
        psB = tpp.tile([P, 4 * CK], F32, name=f"psB{j}", tag="tp")
        for v, row in enumerate((gv, bev, muv, vav)):
            for c in range(CK):
                nc.tensor.matmul(
                    psB[:, v * CK + c : v * CK + c + 1],
                    row[0:1, c * P : (c + 1) * P],
                    ones32[0:1, 0:1],
                )
        bn4 = bnp.tile([P, 4, CK], F32, name=f"bn4_{j}", tag=f"bn4_{j}")
        nc.vector.tensor_copy(bn4[:], psB[:].rearrange("p (v c) -> p v c", c=CK))
        gvp, bevp, muvp, vavp = (bn4[:, v, :] for v in range(4))

        tv = bnp.tile([P, CK], F32, name=f"tv{j}", tag="btmp")
        nc.vector.tensor_scalar_add(tv[:], vavp, BN_EPS)
        rv = bnp.tile([P, CK], F32, name=f"rv{j}", tag="btmp")
        nc.vector.reciprocal(rv[:], tv[:])
        sv = bnp.tile([P, CK], F32, name=f"sv{j}", tag="btmp")
        nc.scalar.activation(sv[:], rv[:], AF.Sqrt)           # rsqrt(var+eps)
        inv = const.tile([P, CK], F32, name=f"inv{j}", tag=f"inv{j}")
        nc.vector.tensor_mul(inv[:], sv[:], gvp)              # gamma * rsqrt
        mi = bnp.tile([P, CK], F32, name=f"mi{j}", tag="btmp")
        nc.vector.tensor_mul(mi[:], muvp, inv[:])
        bv = const.tile([P, CK], F32, name=f"bv{j}", tag=f"bv{j}")
        nc.vector.tensor_sub(bv[:], bevp, mi[:])              # beta - mean*inv
        bvec[j] = bv
        _inv[j] = inv

    def quant_chain(j, c, k):
        """Quantization pipeline for co-chunk c, ci-half k, followed by the
        PE transposes and the Winograd u1/u2 weight combos for that piece."""
        istep, w32 = _istep[j], _w32[j]
        wq = wqp.tile([P, P, TAPS], F16, name=f"wq{j}_{c}_{k}", tag="w16")
        ks = slice(k * P, (k + 1) * P)
        if True:
            src = w32[:, c, ks, :]
            # wl = rne(w * istep); the +-127 clip is redundant: |w*istep|
            # <= 127*(1+2^-23) by construction, and rne of that is 127.
            # Same for the centroid's +-8 clip (|gm|/9/cstep <= 8).
            wlr = whalf.tile([P, P, TAPS], F32, name=f"wlr{j}_{c}_{k}", tag="wh")
            nc.scalar.activation(
                wlr[:], src, AF.Identity, bias=magicv[:, 0:1], scale=istep[:, 0:1]
            )
            # integer levels |wl| <= 127: exact in fp16 (cheaper DVE ops)
            wl3 = whalf.tile([P, P, TAPS], F16, name=f"wl3{j}_{c}_{k}", tag="wh16")
            nc.vector.tensor_scalar_sub(wl3[:], wlr[:], MAGIC)

            # per-grain (co, ci) mean over the 9 taps -> centroid levels
            gm = bnp.tile([P, P], F32, name=f"gm{j}_{c}_{k}", tag="gm")
            nc.vector.tensor_reduce(gm[:], wl3[:], axis=AX.X, op=OP.add)
            c1 = bnp.tile([P, P], F32, name=f"c1{j}_{c}_{k}", tag="c1")
            nc.vector.tensor_scalar(
                c1[:], gm[:], 1.0 / (TAPS * CSTEP), MAGIC, OP.mult, OP.add
            )
            cent = bnp.tile([P, P], F16, name=f"cent{j}_{c}_{k}", tag="cent")
            nc.vector.tensor_scalar(
                cent[:], c1[:], MAGIC, CSTEP, OP.subtract, OP.mult
            )
            centb = cent.unsqueeze(2).broadcast_to((P, P, TAPS))

            # dev = rne(clip(wl - cent, -63.5, 63.5)); wq = dev + cent
            # (all values are k/8 grained, |.|<255: exact in fp16)
            dv = whalf.tile([P, P, TAPS], F16, name=f"dv{j}_{c}_{k}", tag="wh16")
            nc.vector.tensor_sub(dv[:], wl3[:], centb)
            dv2 = whalf.tile([P, P, TAPS], F16, name=f"dv2{j}_{c}_{k}", tag="wh16")
            nc.vector.tensor_scalar(dv2[:], dv[:], DEVW, -DEVW, OP.min, OP.max)
            dv3 = whalf.tile([P, P, TAPS], F16, name=f"dv3{j}_{c}_{k}", tag="wh16")
            nc.vector.tensor_scalar(
                dv3[:], dv2[:], MAGIC, MAGIC, OP.add, OP.subtract
            )
            nc.vector.tensor_add(wq[:], dv3[:], centb)

            # PE-transpose the 9 taps of this (m=c, k): [co,ci] -> [ci,co]
            for t0 in (0, 4, 8):
                nb = min(4, TAPS - t0)
                pst = tpp.tile(
                    [P, nb, P], F16, name=f"pst{j}_{c}_{k}_{t0}", tag="tp"
                )
                for dt in range(nb):
                    nc.tensor.transpose(
                        pst[:, dt, :], wq[:, :, t0 + dt], ident16[:]
                    )
                nc.scalar.copy(wT[j][k][:, c, t0 : t0 + nb, :], pst[:])

        # Winograd combos over dy for this piece: u1/u2 = (g0 +- g1 + g2)/2
        g = wT[j][k]
        s = wsp.tile([P, 3, P], F16, name=f"ws{j}_{c}_{k}", tag="ws")
        nc.vector.tensor_add(s[:], g[:, c, 0:3, :], g[:, c, 6:9, :])
        a = wsp.tile([P, 3, P], F16, name=f"wa{j}_{c}_{k}", tag="ws")
        nc.vector.tensor_add(a[:], s[:], g[:, c, 3:6, :])
        nc.vector.tensor_scalar_mul(wU[j][k][:, c, 0], a[:], 0.5)
        b = wsp.tile([P, 3, P], F16, name=f"wb{j}_{c}_{k}", tag="ws")
        nc.vector.tensor_sub(b[:], s[:], g[:, c, 3:6, :])
        nc.vector.tensor_scalar_mul(wU[j][k][:, c, 1], b[:], 0.5)

    # ---------------- winograd forward transform ---------------------------
    def fwd_v(i, j):
        """v components from the padded source rows (xp for conv1, h for
        conv2): 4 full-width fp16 tensor_tensor ops."""
        src = xp_t[i] if j == 1 else h_t[i]
        v = pvv.tile([P, 4, CK, R2, WP], F16, name=f"v{j}_{i}", tag="v")
        sv = src.rearrange("p c (r two) x -> p c r two x", two=2)

        def d(a):
            return sv[:, :, a // 2 : a // 2 + R2, a % 2, :]

        nc.vector.tensor_sub(v[:, 0], d(0), d(2))
        nc.vector.tensor_add(v[:, 1], d(1), d(2))
        nc.vector.tensor_sub(v[:, 2], d(2), d(1))
        nc.vector.tensor_sub(v[:, 3], d(1), d(3))
        if j == 1:
            v1_t[i] = v
        else:
            v2_t[i] = v

    # ---------------- conv sites -------------------------------------------
    def wsrc(j, u, k, m, dx):
        if u == 0:
            return wT[j][k][:, m, dx, :]
        if u == 3:
            return wT[j][k][:, m, 6 + dx, :]
        return wU[j][k][:, m, u - 1, dx, :]

    def site(j, i, m):
        """One (conv j, image i, co-chunk m) site: 24 MMs into 4 PSUM banks,
        scalar copies with BN scale folded, DVE inverse combine + output."""
        if j == 1 and m == 0:
            hh = phh.tile([P, CK, HP, WP], F16, name=f"h{i}", tag="h")
            nc.gpsimd.memset(hh[:, :, 0:1, :], 0.0)
            nc.gpsimd.memset(hh[:, :, HP - 1 :, :], 0.0)
            nc.gpsimd.memset(hh[:, :, 1 : HP - 1, 0:XO], 0.0)
            nc.gpsimd.memset(hh[:, :, 1 : HP - 1, XO + W :], 0.0)
            h_t[i] = hh
        v = v1_t[i] if j == 1 else v2_t[i]
        pss = {}
        mus = {}
        # u order [0,3,1,2]: u0/u3 need only wT (no wU dependency), so the PE
        # can start before this chunk's u1/u2 combos are done.  For the last
        # image's conv2 sites put u3 last so the even-row half drains early.
        tail = j == 2 and i == BPC - 1
        for u in ((0, 1, 2, 3) if tail else (0, 3, 1, 2)):
            ps = psp.tile([P, NN], F32, name=f"ps{j}_{i}_{m}_{u}", tag="ps")
            idx = 0
            for k in range(CK):
                for dx in range(3):
                    nc.tensor.matmul(
                        ps[:],
                        wsrc(j, u, k, m, dx),
                        v[:, u, k, :, 1 + dx : 1 + dx + W],
                        start=(idx == 0),
                        stop=(idx == 5),
                    )
                    idx += 1
            pss[u] = ps
            if u in (1, 2):
                # only m1/m2 are used twice; scale them out via ScalarE.
                mu = pmu.tile([P, NN], F16, name=f"mu{j}_{i}_{m}_{u}", tag="mu")
                nc.scalar.activation(
                    mu[:], ps[:], AF.Identity, scale=inv_s[j][:, m : m + 1]
                )
                mus[u] = mu
        m1, m2 = mus[1], mus[2]
        # o0 = inv*ps0 + m1 + m2 ; o1 = m1 - m2 - inv*ps3  (fused STT reads
        # PSUM once each for the single-use components)
        t0 = ptt.tile([P, NN], F16, name=f"t0_{j}_{i}_{m}", tag="t")
        nc.vector.scalar_tensor_tensor(
            t0[:], pss[0][:], inv_s[j][:, m : m + 1], m1[:], OP.mult, OP.add
        )
        o0 = ptt.tile([P, NN], F16, name=f"o0_{j}_{i}_{m}", tag="t")
        nc.vector.tensor_add(o0[:], t0[:], m2[:])
        t1 = ptt.tile([P, NN], F16, name=f"t1_{j}_{i}_{m}", tag="t")
        nc.vector.tensor_sub(t1[:], m1[:], m2[:])
        o1 = ptt.tile([P, NN], F16, name=f"o1_{j}_{i}_{m}", tag="t")
        nc.vector.scalar_tensor_tensor(
            o1[:], pss[3][:], ninv_s[j][:, m : m + 1], t1[:], OP.mult, OP.add
        )
        o0v = o0.rearrange("p (r x) -> p r x", x=W)
        o1v = o1.rearrange("p (r x) -> p r x", x=W)
        if j == 1:
            # h rows: even outputs -> padded row 1+2r', odd -> 2+2r'.
            # bias+relu fused into the ScalarE write (DVE stays free).
            hv = h_t[i].rearrange("p c (r two) x -> p c r two x", two=2)
            nc.scalar.activation(
                hv[:, m, 0:R2, 1, XO : XO + W], o0v, AF.Relu,
                bias=bvec[1][:, m : m + 1],
            )
            nc.scalar.activation(
                hv[:, m, 1 : 1 + R2, 0, XO : XO + W], o1v, AF.Relu,
                bias=bvec[1][:, m : m + 1],
            )
        else:
            xbv = xb_t[i].rearrange("p c (r two) x -> p c r two x", two=2)
            z0 = ptt.tile([P, R2, W], F16, name=f"z0_{i}_{m}", tag="t")
            nc.vector.tensor_add(z0[:], o0v, xbv[:, m, :, 0, :])
            z1 = ptt.tile([P, R2, W], F16, name=f"z1_{i}_{m}", tag="t")
            nc.vector.tensor_add(z1[:], o1v, xbv[:, m, :, 1, :])
            yf = pyy.tile([P, H, W], F32, name=f"y{i}_{m}", tag="y")
            yv = yf.rearrange("p (r two) x -> p r two x", two=2)
            nc.scalar.activation(yv[:, :, 0, :], z0[:], AF.Relu)
            nc.scalar.activation(yv[:, :, 1, :], z1[:], AF.Relu)
            nc.gpsimd.dma_start(y_view[i][:, m], yf[:])

    def pe_warmup(n, gated=False):
        """Junk matmuls to hold the PE HAM at K=8/8 through the head's DMA
        wait, so real matmuls start warm.  gated=True makes them depend on
        istep so the scheduler cannot run them before the absmax path."""
        for i in range(n):
            scr_ps = psp.tile([P, NN], F32, name=f"warm{_wuid[0]}", tag="ps")
            _wuid[0] += 1
            if gated:
                rhsb = _istep16[0].broadcast_to((P, 3 * P))
            else:
                rhsb = ident16.unsqueeze(1).broadcast_to((P, 3, P))
            nc.tensor.matmul(scr_ps[:, 0 : 3 * P], ident16[:], rhsb)

    _wuid = [0]
    _istep16 = [None]

    # ---------------- emission order (engine priority) ---------------------
    pe_warmup(36)
    quant_dma(1)
    bn_prep(1)
    bn_prep(2)
    load_dma(0)
    load_dma(1)
    quant_dma(2)
    for i in range(2, BPC):
        load_dma(i)
    prep_x(0)
    prep_x(1)
    quant_absmax(1)
    pe_warmup(44, gated=True)
    quant_chain(1, 0, 0)
    quant_chain(1, 0, 1)
    fwd_v(0, 1)
    site(1, 0, 0)
    pe_warmup(4)
    prep_x(2)
    fwd_v(1, 1)
    site(1, 1, 0)
    pe_warmup(4)
    quant_chain(1, 1, 0)
    fwd_v(2, 1)
    site(1, 2, 0)
    pe_warmup(4)
    quant_chain(1, 1, 1)
    site(1, 0, 1)
    pe_warmup(4)
    site(1, 1, 1)
    pe_warmup(4)
    prep_x(3)
    fwd_v(3, 1)
    site(1, 3, 0)
    pe_warmup(4)
    quant_absmax(2)
    site(1, 2, 1)
    pe_warmup(4)
    quant_chain(2, 0, 0)
    site(1, 3, 1)
    pe_warmup(4)
    prep_x(4)
    fwd_v(4, 1)
    site(1, 4, 0)
    pe_warmup(4)
    quant_chain(2, 0, 1)
    site(1, 4, 1)
    pe_warmup(4)
    fwd_v(0, 2)
    site(2, 0, 0)
    pe_warmup(4)
    prep_x(5)
    fwd_v(5, 1)
    site(1, 5, 0)
    pe_warmup(4)
    site(1, 5, 1)
    pe_warmup(4)
    fwd_v(1, 2)
    site(2, 1, 0)
    prep_x(6)
    fwd_v(6, 1)
    site(1, 6, 0)
    site(1, 6, 1)
    quant_chain(2, 1, 0)
    quant_chain(2, 1, 1)
    site(2, 0, 1)
    prep_x(7)
    fwd_v(7, 1)
    site(1, 7, 0)
    site(1, 7, 1)
    site(2, 1, 1)
    fwd_v(2, 2)
    site(2, 2, 0)
    site(2, 2, 1)
    for i in range(3, BPC):
        fwd_v(i, 2)
        site(2, i, 0)
        site(2, i, 1)


def build_bass():
    nc = bacc.Bacc(
        "TRN2", target_bir_lowering=False, debug=False, num_devices=NCORES
    )
    td = {}
    td["x"] = nc.dram_tensor("x", (BPC, C, H, W), F32, kind="ExternalInput")
    for j in (1, 2):
        td[f"w{j}"] = nc.dram_tensor(f"w{j}", (C, C, 3, 3), F32, kind="ExternalInput")
        for v in ("gamma", "beta", "mean", "var"):
            td[f"{v}{j}"] = nc.dram_tensor(f"{v}{j}", (C,), F32, kind="ExternalInput")
    td["y"] = nc.dram_tensor("y", (BPC, C, H, W), F32, kind="ExternalOutput")

    with tile.TileContext(nc) as tc:
        with ExitStack() as ctx:
            _emit(nc, tc, ctx, td)
    nc.compile()
    return nc


_NC = None


def _get_nc():
    global _NC
    if _NC is None:
        _NC = build_bass()
    return _NC


def make_in_maps(x, w1, gamma1, beta1, mean1, var1, w2, gamma2, beta2, mean2, var2):
    rep = {
        "w1": w1, "gamma1": gamma1, "beta1": beta1, "mean1": mean1, "var1": var1,
        "w2": w2, "gamma2": gamma2, "beta2": beta2, "mean2": mean2, "var2": var2,
    }
    rep = {k: np.ascontiguousarray(np.asarray(v), dtype=np.float32) for k, v in rep.items()}
    in_maps = []
    for c in range(NCORES):
        m = {"x": np.ascontiguousarray(np.asarray(x)[c * BPC : (c + 1) * BPC], dtype=np.float32)}
        m.update(rep)
        in_maps.append(m)
    return in_maps


def kernel(x, w1, gamma1, beta1, mean1, var1,
           w2, gamma2, beta2, mean2, var2, codebook=None, **_unused):
    nc = _get_nc()
    in_maps = make_in_maps(x, w1, gamma1, beta1, mean1, var1,
                           w2, gamma2, beta2, mean2, var2)
    res = run_bass_kernel_spmd(nc, in_maps, core_ids=list(range(NCORES)))
    return np.concatenate([r["y"] for r in res.results], axis=0)
